# revision 19
# baseline (speedup 1.0000x reference)
"""Attention graph convolution (GAT layer) on 8 TRN2 NeuronCores.

Reference computation (all fp32):
    h   = input @ W                      # (N, 64)
    e   = leakyrelu(h@a1 + (h@a2).T)     # (N, N)
    att = softmax(where(adj>0, e, -inf)) # row softmax
    out = elu(att @ h)                   # (N, 64)

Sharding: rows of e/att (= output rows) are split across 8 cores,
1536 rows each.  h (N x 64) is computed on every core (tiny).

Host-side prep (layout only, byte counts unchanged): each core gets its
adjacency row-block TRANSPOSED (adjt[j, i] = adj[own_i, j]) and the
input transposed (inp_t = input.T), so the SWDGE cast DMA lands adj.T
directly in the [j partitions, i free] layout the PE contraction needs
and h is computed without any on-device transposes.

Per-core algorithm (core owns rows I, |I| = 1536):
  - no max-subtraction softmax: |z| < ~30 so U = adj.T * exp(lrelu(z))
    cannot overflow and equals the reference numerator up to a common
    row scale (cancelled by the final division).
  - denominator via ones-column: P.T = [h | 1].T @ U.T;
    out = elu(P[:, :64] / P[:, 64]).
  - t = lrelu(z)/C is computed in fp16 (C=40) so exp can be evaluated
    as exp(C*t + bias) on ACT with 2-byte tiles throughout.  Most
    windows: plain SWDGE window DMA of adj.T (bf16) + one batched DVE
    multiply (2x mode) applies the mask.  `nfold` windows instead fold
    the mask into the DMA itself: accum_op=add lands adj.T ON TOP of t
    and exp(C*t - C) zeroes masked entries (adj=0 terms come out
    <= e^-10 relative to any real term) -- trading DVE time for DMA
    time, tunable to balance the two.
  - leakyrelu/C as two 4x tensor_scalar ops + a 2x tensor_tensor max,
    or Prelu on ACT for `act_frac` of chunks (schedule-balanced).
  - accumulation matmuls are bf16 (1 cycle/row vs fp32's 4).
"""

import numpy as np

N_TOTAL = 12288
K_IN = 128
F_OUT = 64
N_CORES = 8
ALPHA = 0.2
CEXP = 40.0


def build_program(
    nt: int,          # total nodes (rows of adjt)
    no: int,          # nodes owned by this core (cols of adjt)
    jw: int,          # j window size (adj.T rows resident in SBUF at once)
    act_frac: float = 0.41,   # fraction of j chunks with leakyrelu on ACT
    nfold: int = 0,           # windows using the DMA-accum mask fold
):
    from contextlib import ExitStack

    import concourse.bass as bass
    import concourse.mybir as mybir
    import concourse.tile as tile
    from concourse import bacc
    from concourse.alu_op_type import AluOpType

    f32 = mybir.dt.float32
    i32 = mybir.dt.int32
    bf16 = mybir.dt.bfloat16
    fp16 = mybir.dt.float16
    AF = mybir.ActivationFunctionType

    P = 128
    F = F_OUT
    FE = F + 1                    # h columns + ones column
    K = K_IN
    assert nt % P == 0 and no % P == 0 and jw % P == 0 and nt % jw == 0
    ncj = nt // P                 # j chunks
    nw = nt // jw                 # windows
    cpw = jw // P                 # j chunks per window
    nic = no // P                 # i chunks (own rows)
    S = 512                       # i split for matmul N-dim / psum banks
    ns = no // S
    assert no % S == 0
    IC = 1.0 / CEXP

    # fold windows, spread over [2, nw)
    fold_ws = set()
    if nfold > 0:
        step = max(1, (nw - 2) // nfold)
        w = nw - 1
        while len(fold_ws) < min(nfold, nw - 2) and w >= 2:
            fold_ws.add(w)
            w -= step

    nc = bacc.Bacc("TRN2", target_bir_lowering=False, debug=False,
                   num_devices=1)

    inp_t = nc.dram_tensor("inp_t", [K, nt], f32, kind="ExternalInput")
    inp_own_t = nc.dram_tensor("inp_own_t", [K, no], f32,
                               kind="ExternalInput")
    adjt = nc.dram_tensor("adjt", [nt, no], i32, kind="ExternalInput")
    w_d = nc.dram_tensor("W", [K, F], f32, kind="ExternalInput")
    a_d = nc.dram_tensor("a", [2 * F, 1], f32, kind="ExternalInput")
    out_d = nc.dram_tensor("out", [no, F], f32, kind="ExternalOutput")

    # adj.T rows as [partition, chunk, i]: row (n*128 + p) -> [p, n]
    adjt_r = adjt.ap().rearrange("(n p) i -> p n i", p=P)

    with tile.TileContext(nc) as tc, ExitStack() as ctx:
        consts = ctx.enter_context(tc.tile_pool(name="consts", bufs=1))
        scr_ps = ctx.enter_context(
            tc.tile_pool(name="scr_ps", bufs=1, space="PSUM"))
        p1b_ps = ctx.enter_context(
            tc.tile_pool(name="p1b_ps", bufs=2, space="PSUM"))
        adjw_pool = ctx.enter_context(tc.tile_pool(name="adjw", bufs=2))

        # ---- plain adj.T window DMAs for the first windows so the SWDGE
        # stream (the critical resource) starts at t=0.
        adjw_tiles = {}

        def issue_adjw(w):
            # window 0 in per-pair slices so the first mask starts ~5us in;
            # later windows as one DMA (less SWDGE descriptor-gen time)
            t = adjw_pool.tile([P, cpw, no], bf16, tag="adjw",
                               name=f"adjw_{w}")
            if w == 0:
                for jp in range(cpw // 2):
                    c0 = 2 * jp
                    nc.gpsimd.dma_start(t[:, 2 * jp:2 * jp + 2, :],
                                        adjt_r[:, c0:c0 + 2, :])
            else:
                nc.gpsimd.dma_start(t[:],
                                    adjt_r[:, w * cpw:(w + 1) * cpw, :])
            adjw_tiles[w] = t

        issue_adjw(0)
        issue_adjw(1)

        # ---- phase 0: Wa1 = W @ a1, Wa2 = W @ a2 -------------------------
        wwa2_sb = consts.tile([K, FE], f32)    # [Wa2 | W] (128 x 65)
        nc.sync.dma_start(wwa2_sb[:, 1:FE], w_d.ap())
        a_row = consts.tile([1, 2 * F], f32)   # a as a single-partition row
        nc.sync.dma_start(a_row[:], a_d.ap().rearrange("n o -> o n"))
        io_t = consts.tile([K, no], f32)       # input_own.T
        nc.sync.dma_start(io_t[:], inp_own_t.ap())


        ones_sb = consts.tile([P, P], f32)
        nc.vector.memset(ones_sb[:], 1.0)
        negc_sb = consts.tile([P, 1], f32)     # -C bias for the masked exp
        nc.vector.memset(negc_sb[:], -CEXP)
        # replicate a across partitions via a K=1 matmul with a ones row
        a_rep = consts.tile([P, 2 * F], f32)
        a_rep_ps = scr_ps.tile([P, 2 * F], f32, tag="scr")
        nc.tensor.matmul(a_rep_ps[:], ones_sb[0:1, :], a_row[:],
                         start=True, stop=True)
        nc.vector.tensor_copy(a_rep[:], a_rep_ps[:])

        wa12_sb = consts.tile([K, 2], f32)
        wtmp = consts.tile([K, F], f32)
        nc.vector.tensor_tensor(wtmp[:], wwa2_sb[:, 1:FE], a_rep[:, 0:F],
                                AluOpType.mult)
        nc.vector.tensor_reduce(wa12_sb[:, 0:1], wtmp[:],
                                mybir.AxisListType.X, AluOpType.add)
        nc.vector.tensor_tensor(wtmp[:], wwa2_sb[:, 1:FE], a_rep[:, F:2 * F],
                                AluOpType.mult)
        nc.vector.tensor_reduce(wa12_sb[:, 1:2], wtmp[:],
                                mybir.AxisListType.X, AluOpType.add)
        nc.vector.tensor_copy(wwa2_sb[:, 0:1], wa12_sb[:, 1:2])
        wa1_rep = consts.tile([K, P], f32)     # Wa1 replicated to 128 cols
        nc.vector.tensor_scalar(wa1_rep[:], ones_sb[:], wa12_sb[:, 0:1], None,
                                AluOpType.mult)

        # ---- phase 1a: Wh1_rep[p, x] = Wh1[own x] for all p (fp16) -------
        wh1_h = consts.tile([P, no], fp16)
        for s in range(ns):
            w1p = scr_ps.tile([P, S], f32, tag="scr")
            nc.tensor.matmul(w1p[:], wa1_rep[:], io_t[:, s * S:(s + 1) * S],
                             start=True, stop=True)
            nc.vector.tensor_copy(wh1_h[:, s * S:(s + 1) * S], w1p[:])

        # ---- phase 1b (emitted interleaved below): h_ext, Wh2 ------------
        FE2 = FE + 1                  # [wh2 | h | ones]
        h_ext = consts.tile([P, ncj, FE2], bf16)
        wh2_sb = consts.tile([P, ncj], f32)    # Wh2 (f32 scalars for DVE)
        wh2c_sb = consts.tile([P, ncj], f32)   # Wh2 / C for ACT Prelu bias
        nc.vector.memset(h_ext[:, :, FE], 1.0)

        def phase1b_chunk(jc, tp, q):
            # h_ext[:, jc, :] = input[chunk jc] @ [W | Wa2]; tp is input.T
            # for this window
            hw_ps = p1b_ps.tile([P, FE], f32, tag="p1b")
            nc.tensor.matmul(hw_ps[:], tp[:, q * P:(q + 1) * P],
                             wwa2_sb[:], start=True, stop=True)
            nc.scalar.copy(h_ext[:, jc, 0:FE], hw_ps[:])

        # ---- phase 2: main loop over j windows / j chunks ----------------
        pt_pool = ctx.enter_context(
            tc.tile_pool(name="pt_acc", bufs=1, space="PSUM"))
        pt_ps = pt_pool.tile([FE, no], f32)

        n_act = int(round(act_frac * ncj))

        def lrelu_engine(jc):
            return "act" if (jc * 7919) % ncj < n_act else "dve"

        def lrelu_chunk(jc, dst, scr):
            # dst = lrelu(Wh1 + Wh2) / C   in fp16
            if lrelu_engine(jc) == "act":
                nc.scalar.activation(dst, wh1_h[:], AF.Prelu,
                                     bias=wh2c_sb[:, jc:jc + 1],
                                     scale=IC, alpha=ALPHA)
                return
            # t1 = z*0.2/C ; t2 = z/C ; dst = max(t1, t2)
            nc.vector.tensor_scalar(scr, wh1_h[:],
                                    wh2_sb[:, jc:jc + 1], ALPHA * IC,
                                    AluOpType.add, AluOpType.mult)
            nc.vector.tensor_scalar(dst, wh1_h[:],
                                    wh2_sb[:, jc:jc + 1], IC,
                                    AluOpType.add, AluOpType.mult)
            nc.vector.tensor_tensor(dst, dst, scr, AluOpType.max)

        npair = cpw // 2
        assert cpw % 2 == 0

        with (
            tc.tile_pool(name="wt", bufs=3) as wt_pool,
            tc.tile_pool(name="tin", bufs=3) as tin_pool,
            tc.tile_pool(name="lscr", bufs=1) as l_pool,
            tc.tile_pool(name="epool", bufs=2) as e_pool,
            tc.tile_pool(name="upool", bufs=3) as u_pool,
        ):
            wt_tiles = {}
            next_prep = 0

            def prep_window(wp):
                # window DMA first (no dependencies) so the SWDGE stream
                # never waits behind the copies on the gpsimd queue; then
                # phase 1b and leakyrelu into the window t-tile.
                if wp >= 2 and wp not in fold_ws:
                    issue_adjw(wp)
                tp = tin_pool.tile([K, jw], f32, tag="tin")
                nc.sync.dma_start(tp[:], inp_t.ap()[:, wp * jw:(wp + 1) * jw])
                for q in range(cpw):
                    phase1b_chunk(wp * cpw + q, tp, q)
                cs = slice(wp * cpw, (wp + 1) * cpw)
                nc.vector.tensor_copy(wh2_sb[:, cs], h_ext[:, cs, 0])
                nc.vector.tensor_scalar(wh2c_sb[:, cs], h_ext[:, cs, 0],
                                        IC, None, AluOpType.mult)
                wt = wt_pool.tile([P, cpw, no], fp16, tag="wt",
                                  name=f"wt_{wp}")
                for q in range(cpw):
                    jc = wp * cpw + q
                    scr = l_pool.tile([P, no], fp16, tag="lscr")
                    lrelu_chunk(jc, wt[:, q, :], scr[:])
                if wp in fold_ws:
                    nc.gpsimd.dma_start(
                        wt[:], adjt_r[:, wp * cpw:(wp + 1) * cpw, :],
                        accum_op=AluOpType.add)
                wt_tiles[wp] = wt

            for w in range(nw):
                while next_prep <= min(w + 2, nw - 1):
                    prep_window(next_prep)
                    next_prep += 1
                wt = wt_tiles.pop(w)
                folded = w in fold_ws
                adjw = None if folded else adjw_tiles.pop(w)
                for jp in range(npair):
                    jcs = [w * cpw + 2 * jp, w * cpw + 2 * jp + 1]
                    u_sb = u_pool.tile([P, 2, no], bf16, tag="u")
                    if folded:
                        # U = exp(C*t - C) = adj * exp(lrelu(z))
                        nc.scalar.activation(
                            u_sb[:], wt[:, 2 * jp:2 * jp + 2, :],
                            AF.Exp, bias=negc_sb[:], scale=CEXP)
                    else:
                        # E = exp(C*t); U = E * adj (bf16 2x DVE)
                        e_sb = e_pool.tile([P, 2, no], bf16, tag="e")
                        nc.scalar.activation(
                            e_sb[:], wt[:, 2 * jp:2 * jp + 2, :],
                            AF.Exp, scale=CEXP)
                        nc.vector.tensor_tensor(
                            u_sb[:], e_sb[:], adjw[:, 2 * jp:2 * jp + 2, :],
                            AluOpType.mult)
                    for q, jc in enumerate(jcs):
                        for s in range(ns):
                            nc.tensor.matmul(
                                pt_ps[:, s * S:(s + 1) * S],
                                h_ext[:, jc, 1:FE2],
                                u_sb[:, q, s * S:(s + 1) * S],
                                start=(jc == 0),
                                stop=(jc == ncj - 1))

        # ---- phase 3: out = elu(P[:, :64] / P[:, 64]) --------------------
        from concourse.masks import make_identity
        with tc.tile_pool(name="fin_c", bufs=1) as fin_c, \
                tc.tile_pool(name="fin_sb", bufs=4) as fin_sb:
            identity = fin_c.tile([P, P], f32)
            make_identity(nc, identity)
            pt_sb = fin_c.tile([FE, no], f32)
            nc.vector.tensor_copy(pt_sb[:], pt_ps[:])
            for ic in range(nic):
                ptp = p1b_ps.tile([P, FE], f32, tag="p1b")
                nc.tensor.transpose(ptp[:], pt_sb[:, ic * P:(ic + 1) * P],
                                    identity[0:FE, 0:FE])
                rec = fin_sb.tile([P, 1], f32, tag="rec")
                nc.vector.reciprocal(rec[:], ptp[:, F:FE])
                hp = fin_sb.tile([P, F], f32, tag="hp")
                nc.vector.tensor_scalar(hp[:], ptp[:, 0:F], rec[:], None,
                                        AluOpType.mult)
                # elu(x) = max(x,0) + exp(min(x,0)) - 1
                mn = fin_sb.tile([P, F], f32, tag="mn")
                nc.vector.tensor_scalar(mn[:], hp[:], 0.0, None, AluOpType.min)
                nc.scalar.activation(mn[:], mn[:], AF.Exp)
                nc.vector.tensor_scalar(hp[:], hp[:], 0.0, None, AluOpType.max)
                ob = fin_sb.tile([P, F], f32, tag="ob")
                nc.vector.scalar_tensor_tensor(
                    ob[:], mn[:], 1.0, hp[:],
                    AluOpType.subtract, AluOpType.add)
                nc.sync.dma_start(out_d[ic * P:(ic + 1) * P, :], ob[:])

    nc.compile()
    return nc


_CACHE = {}


def _get_program(nt, no, jw, **kw):
    key = (nt, no, jw, tuple(sorted(kw.items())))
    if key not in _CACHE:
        _CACHE[key] = build_program(nt, no, jw, **kw)
    return _CACHE[key]


def make_in_maps(input, adj, W, a):
    input = np.ascontiguousarray(input, dtype=np.float32)
    adj = np.ascontiguousarray(adj, dtype=np.int32)
    W = np.ascontiguousarray(W, dtype=np.float32)
    a = np.ascontiguousarray(a, dtype=np.float32)
    nt = input.shape[0]
    no = nt // N_CORES
    inp_t = np.ascontiguousarray(input.T)
    in_maps = []
    for c in range(N_CORES):
        in_maps.append({
            "inp_t": inp_t,
            "inp_own_t": np.ascontiguousarray(inp_t[:, c * no:(c + 1) * no]),
            "adjt": np.ascontiguousarray(adj[c * no:(c + 1) * no].T),
            "W": W,
            "a": a,
        })
    return in_maps


def kernel(input, adj, W, a):
    from concourse.bass_utils import run_bass_kernel_spmd

    nt = input.shape[0]
    no = nt // N_CORES
    nc = _get_program(nt, no, 1024)
    in_maps = make_in_maps(input, adj, W, a)
    res = run_bass_kernel_spmd(nc, in_maps, list(range(N_CORES)))
    return np.concatenate([r["out"] for r in res.results], axis=0)
